# revision 4
# baseline (speedup 1.0000x reference)
"""Trainium2 Bass kernel for nn_BareDotProdAttnEncoder (tree scan, gnn_message_passing).

Reference semantics (per batch element b):
  h_0 = x_0
  for i in 1..N-1:
      p = parent[i]  (p < i)
      alpha = exp(<h_p, x_i>); beta = exp(<x_i, x_i>)
      h_i = (alpha*h_p + beta*x_i) / (alpha + beta + 1e-15)

Equivalent form used on device:
  w = sigmoid(<h_p, x_i> - <x_i, x_i>)      (= alpha/(alpha+beta))
  h_i = w*(h_p - x_i) + x_i

Since depth[i] = depth[parent[i]] + 1, all parents of level-l nodes are at
level l-1: the scan is a level-by-level sweep (L ~ 18 levels for N=2048).
The host computes the level schedule, sorts nodes level-contiguously, and
pre-permutes the embeddings into that order, so the device does:
  per level: contiguous X load (prefetchable), SWDGE index-gather of parent
  h from the persistent HBM state buffer, the dot/sigmoid/blend math, and
  contiguous writebacks of the level into the state buffer (which is also
  the kernel output, inverse-permuted host-side).

All device data is fp16 (tolerance 2e-2; fp16 keeps rel err ~1e-3), with
fp32 dot-product accumulation.

Sharding: pure data parallelism over the batch; each of the 8 cores owns
4 trees, processed as STREAMS independent streams that pipeline against
each other.
"""

import os
import numpy as np

N_CORES = 8
STREAMS = int(os.environ.get("K_STREAMS", "2"))
TREES_PER_STREAM = 4 // STREAMS
DIM = 512
PART = 128
N_NODE = 2048
REPEAT = int(os.environ.get("K_REPEAT", "1"))
DYN = os.environ.get("K_DYN", "1") == "1"  # dynamic gather counts (skip pad traffic)
SINGLE_PACKET = os.environ.get("K_SINGLEPKT", "1") == "1"
XBUFS = int(os.environ.get("K_XBUFS", "3"))
HBUFS = int(os.environ.get("K_HBUFS", "2"))
F16 = os.environ.get("K_F16", "1") == "1"
DOT = os.environ.get("K_DOT", "stt")       # stt | altact (alternate stt / Act-accum)
BLEND = os.environ.get("K_BLEND", "act")   # stt | act
QFUSE = os.environ.get("K_QFUSE", "1") == "1"  # host |x|^2 as sigmoid bias
PREP = os.environ.get("K_PREP", "1") == "1"    # prepared+triggered gathers
WBSPLIT = int(os.environ.get("K_WBSPLIT", "2"))  # writebacks per level
STAGGER = os.environ.get("K_STAGGER", "0") == "1"
ABLATE = os.environ.get("K_ABLATE", "")  # nodep: gather from fake tensor
XLAYT = os.environ.get("K_XLAYT", "1") == "1"  # xsort in [128, sumC*512] layout


def _compute_depths(conn):
    B, N = conn.shape
    depths = np.zeros((B, N), np.int32)
    bidx = np.arange(B)
    for i in range(1, N):
        depths[:, i] = depths[bidx, conn[:, i]] + 1
    return depths


def _assign_trees(S, B):
    """Group trees into (stream, core) slots to minimize total padded chunks.
    S: per-tree level-size matrix [B, L]. Returns groups[g][c] = tuple of trees.
    Deterministic local search (seeded)."""
    L = S.shape[1]
    tps = TREES_PER_STREAM
    nslots = B // tps  # STREAMS * N_CORES
    nat = [tuple(range(tps * s, tps * (s + 1))) for s in range(nslots)]
    if tps == 4:
        return [[nat[g * N_CORES + c] for c in range(N_CORES)] for g in range(STREAMS)]

    def cost(assign):
        tot = 0
        for g in range(STREAMS):
            lv = np.zeros(L, np.int64)
            for c in range(N_CORES):
                grp = assign[g * N_CORES + c]
                n = np.sum(S[list(grp)], axis=0)
                lv = np.maximum(lv, (n + PART - 1) // PART)
            tot += lv.sum()
        return int(tot)

    rng = np.random.default_rng(12345)
    cur = [list(p) for p in nat]
    cc = cost([tuple(p) for p in cur])
    best, bc = [tuple(p) for p in cur], cc
    for _ in range(20000):
        a = int(rng.integers(0, nslots)); b2 = int(rng.integers(0, nslots))
        if a == b2:
            continue
        i = int(rng.integers(0, tps)); j = int(rng.integers(0, tps))
        cur[a][i], cur[b2][j] = cur[b2][j], cur[a][i]
        c2 = cost([tuple(p) for p in cur])
        if c2 <= cc:
            cc = c2
            if c2 < bc:
                best, bc = [tuple(p) for p in cur], c2
        else:
            cur[a][i], cur[b2][j] = cur[b2][j], cur[a][i]
    return [[best[g * N_CORES + c] for c in range(N_CORES)] for g in range(STREAMS)]


def _build_schedule(conn):
    """Host-side schedule: level structure, per-core index arrays, maps.

    Returns (L, Cls, sched) where
      L: number of levels
      Cls[g]: list of per-level chunk counts (uniform across cores)
      sched[c]: dict with per-core input arrays + posmat for assembly
    """
    B, N = conn.shape
    depths = _compute_depths(conn)
    L = int(depths.max()) + 1

    # node lists per (batch, level), ordered by node id (stable)
    order = [[np.nonzero(depths[b] == l)[0] for l in range(L)] for b in range(B)]

    S = np.zeros((B, L), np.int64)
    for b in range(B):
        S[b] = np.bincount(depths[b], minlength=L)
    groups = _assign_trees(S, B)  # groups[g][c] = tree tuple

    # uniform chunk capacities per stream
    Cls = []
    for g in range(STREAMS):
        Cl = np.zeros(L, np.int64)
        for c in range(N_CORES):
            trees = groups[g][c]
            for l in range(L):
                n = sum(len(order[b][l]) for b in trees)
                Cl[l] = max(Cl[l], (n + PART - 1) // PART)
        Cls.append([int(x) for x in Cl])

    sched = []
    for c in range(N_CORES):
        entry = {}
        for g in range(STREAMS):
            Cl = Cls[g]
            sumC = sum(Cl)
            R = PART * sumC
            trees = groups[g][c]
            pad = np.int16(-1 if DYN else 0)
            eidx = np.full(R, -1, np.int32)     # row -> embedding row (t*N + i)
            pidx = np.full(R, pad, np.int16)    # row -> parent state row
            cnt = np.zeros(L, np.int32)         # real rows per level (min 1)
            posmat = np.zeros((TREES_PER_STREAM, N), np.int32)  # node -> state row
            off = 0
            for l in range(L):
                base = PART * off
                j = 0
                for t, b in enumerate(trees):
                    for i in order[b][l]:
                        row = base + j
                        eidx[row] = t * N + i
                        posmat[t, i] = row
                        if l > 0:
                            pidx[row] = posmat[t, conn[b, i]]
                        j += 1
                assert j <= PART * Cl[l]
                if j == 0 and Cl[l] > 0:
                    pidx[base] = 0
                    j = 1
                cnt[l] = j
                off += Cl[l]

            def wrap(vals):
                # gather index layout: within a call of num_idxs n, index j
                # lives at [j%16, j//16]; replicate across the 8 groups of
                # 16 partitions. Calls slice per-level column blocks.
                out = np.zeros((PART, 8 * sumC), np.int16)
                o = 0
                for l in range(L):
                    n = PART * Cl[l]
                    block = vals[PART * o : PART * o + n].reshape(8 * Cl[l], 16).T
                    for rep in range(8):
                        out[16 * rep : 16 * (rep + 1), 8 * o : 8 * (o + Cl[l])] = block
                    o += Cl[l]
                return out

            entry[f"pidx{g}"] = wrap(pidx)
            entry[f"eidx{g}"] = eidx  # flat, for host-side emb permutation
            entry[f"cnt{g}"] = cnt.reshape(1, L)
            entry[f"posmat{g}"] = posmat
            entry[f"trees{g}"] = list(trees)
        sched.append(entry)
    return L, Cls, sched


def _build_program(L, Cls):
    import concourse.bacc as bacc
    import concourse.mybir as mybir
    import concourse.tile as tile

    fdt = mybir.dt.float16 if F16 else mybir.dt.float32
    f32 = mybir.dt.float32
    i16 = mybir.dt.int16
    i32 = mybir.dt.int32
    Alu = mybir.AluOpType
    Act = mybir.ActivationFunctionType

    nqueues = min(STREAMS, 4) if PREP else 1
    nc = bacc.Bacc("TRN2", debug=False, num_swdge_queues=nqueues)

    xsort_t, pidx_t, cnt_t, state_t, negq_t = [], [], [], [], []
    for g in range(STREAMS):
        sumC = sum(Cls[g])
        R = PART * sumC
        xshape = [PART, sumC * DIM] if XLAYT else [R, DIM]
        xsort_t.append(nc.dram_tensor(f"xsort{g}", xshape, fdt,
                                      kind="ExternalInput"))
        pidx_t.append(nc.dram_tensor(f"pidx{g}", [PART, 8 * sumC], i16,
                                     kind="ExternalInput"))
        cnt_t.append(nc.dram_tensor(f"cnt{g}", [1, L], i32, kind="ExternalInput"))
        if QFUSE:
            negq_t.append(nc.dram_tensor(f"negq{g}", [PART, sumC], f32,
                                         kind="ExternalInput"))
        state_t.append(nc.dram_tensor(f"state{g}", [R, DIM], fdt,
                                      kind="ExternalOutput"))
    fake_t = None
    if ABLATE == "nodep":
        Rmax = max(PART * sum(C) for C in Cls)
        fake_t = nc.dram_tensor("fake", [Rmax, DIM], fdt)

    with tile.TileContext(nc) as tc:
        from contextlib import ExitStack
        stack = ExitStack()
        pools = []
        for g in range(STREAMS):
            p = {
                "X": stack.enter_context(tc.tile_pool(name=f"X{g}", bufs=XBUFS)),
                "P": stack.enter_context(tc.tile_pool(name=f"P{g}", bufs=2)),
                "D": stack.enter_context(tc.tile_pool(name=f"D{g}", bufs=2)),
                "T": stack.enter_context(tc.tile_pool(name=f"T{g}", bufs=2)),
                "H": stack.enter_context(tc.tile_pool(name=f"H{g}", bufs=HBUFS)),
                "M": stack.enter_context(tc.tile_pool(name=f"M{g}", bufs=2)),
                "S": stack.enter_context(tc.tile_pool(name=f"S{g}", bufs=2)),
                "I": stack.enter_context(tc.tile_pool(name=f"I{g}", bufs=1)),
            }
            pools.append(p)

        # preload index arrays, allocate junk tiles
        idxs = []
        for g in range(STREAMS):
            sumC = sum(Cls[g])
            pi = pools[g]["I"].tile([PART, 8 * sumC], i16, tag=f"pi{g}")
            nq = None
            if QFUSE:
                nq = pools[g]["I"].tile([PART, sumC], f32, tag=f"nq{g}")
                nc.sync.dma_start(nq[:, :], negq_t[g][:, :])
            jt = pools[g]["I"].tile([PART, DIM], fdt, tag=f"jt{g}")   # dot junk out
            ja = pools[g]["I"].tile([PART, DIM], fdt, tag=f"ja{g}")   # act junk out
            nc.sync.dma_start(pi[:, :], pidx_t[g][:, :])
            cr = None
            if DYN:
                ct = pools[g]["I"].tile([1, L], i32, tag=f"ct{g}")
                nc.sync.dma_start(ct[:, :], cnt_t[g][:, :])
                # one register per level: reusing one would be a WAR hazard
                # under Tile reordering (gather reads reg at exec time)
                regs = [nc.gpsimd.alloc_register(f"cnt{g}_{l}") for l in range(L)]
                cr = (ct, regs)
            sems = None
            if PREP:
                sems = [nc.alloc_semaphore(f"gsem{g}_{l}") for l in range(L)]
            idxs.append((pi, nq, jt, ja, cr, sems))

        for _rep in range(REPEAT):
            offs = [0 for _ in range(STREAMS)]
            if STAGGER:
                emit_order = []
                for w in range(L + STREAMS - 1):
                    for g in range(STREAMS):
                        l = w - g
                        if 0 <= l < L:
                            emit_order.append((l, g))
            else:
                emit_order = [(l, g) for l in range(L) for g in range(STREAMS)]

            # per-stream pending prepared-gather (P tile, level)
            pend = [None for _ in range(STREAMS)]

            def load_nreg(g, l):
                pi, nq, jt, ja, cr, sems = idxs[g]
                if DYN:
                    ct, regs = cr
                    nc.gpsimd.reg_load(regs[l], ct[0:1, l : l + 1])
                    return regs[l]
                return PART * Cls[g][l]

            def emit_prep(g, l, off):
                C = Cls[g][l]
                pi, nq, jt, ja, cr, sems = idxs[g]
                nreg = load_nreg(g, l)
                P = pools[g]["P"].tile([PART, C, DIM], fdt, tag=f"P{g}")
                nc.gpsimd.dma_gather(
                    P[:, :, :], state_t[g][:, :],
                    pi[:, 8 * off : 8 * (off + C)], PART * C, nreg, DIM,
                    single_packet=SINGLE_PACKET,
                    prepare_only=True, sem=sems[l], queue_num=g % 4)
                return P

            for l, g in emit_order:
                C = Cls[g][l]
                if C == 0:
                    continue
                off = offs[g]
                offs[g] += C
                pi, nq, jt, ja, cr, sems = idxs[g]
                p = pools[g]
                n = PART * C

                X = p["X"].tile([PART, C, DIM], fdt, tag=f"X{g}")
                H = p["H"].tile([PART, C, DIM], fdt, tag=f"H{g}")

                if XLAYT:
                    xsrc = xsort_t[g][:, DIM * off : DIM * (off + C)].rearrange(
                        "p (c e) -> p c e", e=DIM)
                else:
                    xsrc = xsort_t[g][PART * off : PART * (off + C)].rearrange(
                        "(c p) e -> p c e", p=PART)
                nc.sync.dma_start(X[:, :, :], xsrc)

                if l == 0:
                    nc.scalar.activation(H[:, :, :], X[:, :, :], Act.Copy)
                else:
                    if PREP:
                        assert pend[g] is not None and pend[g][1] == l
                        P = pend[g][0]
                        pend[g] = None
                        nc.gpsimd.trigger_dma(count=None, queue_num=g % 4)
                    else:
                        nreg = load_nreg(g, l)
                        P = p["P"].tile([PART, C, DIM], fdt, tag=f"P{g}")
                        gsrc = fake_t if ABLATE == "nodep" else state_t[g]
                        nc.gpsimd.dma_gather(
                            P[:, :, :], gsrc[:, :],
                            pi[:, 8 * off : 8 * (off + C)], n, nreg, DIM,
                            single_packet=SINGLE_PACKET)

                    D = p["D"].tile([PART, C, DIM], fdt, tag=f"D{g}")
                    T = None
                    if BLEND == "act":
                        T = p["T"].tile([PART, C, DIM], fdt, tag=f"T{g}")
                    dp = p["S"].tile([PART, C], f32, tag=f"dp{g}")
                    wh = p["S"].tile([PART, C], f32, tag=f"wh{g}")

                    for k in range(C):
                        # D = h_p - x
                        nc.vector.tensor_tensor(D[:, k, :], P[:, k, :], X[:, k, :],
                                                Alu.subtract)
                        # z_k: QFUSE -> <x, h_p> (q in sigmoid bias);
                        #      else  -> <x, h_p - x>
                        din = P[:, k, :] if QFUSE else D[:, k, :]
                        use_act = (DOT == "altact") and (k % 2 == 1)
                        if use_act:
                            M = p["M"].tile([PART, DIM], fdt, tag=f"M{g}")
                            nc.vector.tensor_tensor(M[:, :], X[:, k, :], din,
                                                    Alu.mult)
                            nc.scalar.activation(ja[:, :], M[:, :], Act.Copy,
                                                 accum_out=dp[:, k : k + 1])
                        else:
                            nc.vector.scalar_tensor_tensor(
                                jt[:, :], X[:, k, :], 0.0, din,
                                Alu.bypass, Alu.mult,
                                accum_out=dp[:, k : k + 1])
                        # w = sigmoid(z - |x|^2)
                        if QFUSE:
                            nc.scalar.activation(wh[:, k : k + 1],
                                                 dp[:, k : k + 1], Act.Sigmoid,
                                                 bias=nq[:, off + k : off + k + 1])
                        else:
                            nc.scalar.activation(wh[:, k : k + 1],
                                                 dp[:, k : k + 1], Act.Sigmoid)
                        if BLEND == "act":
                            # T = w * D (Act), h = T + x (DVE 2x tt)
                            nc.scalar.activation(T[:, k, :], D[:, k, :], Act.Copy,
                                                 scale=wh[:, k : k + 1])
                            nc.vector.tensor_tensor(H[:, k, :], T[:, k, :],
                                                    X[:, k, :], Alu.add)
                        else:
                            # h = w*D + x in one DVE stt (1x)
                            nc.vector.scalar_tensor_tensor(
                                H[:, k, :], D[:, k, :], wh[:, k : k + 1],
                                X[:, k, :], Alu.mult, Alu.add)

                # writeback in WBSPLIT pieces so the tail transfer is short
                nsp = min(WBSPLIT, C)
                bounds = [round(i * C / nsp) for i in range(nsp + 1)]
                for i in range(nsp):
                    a, b = bounds[i], bounds[i + 1]
                    if a == b:
                        continue
                    dst = state_t[g][PART * (off + a) : PART * (off + b)].rearrange(
                        "(c p) e -> p c e", p=PART)
                    nc.sync.dma_start(dst, H[:, a:b, :])

                # prep the NEXT level's gather now (desc-gen overlaps this
                # level's DMA/compute; the data dep rides the trigger)
                if PREP:
                    lnext = l + 1
                    while lnext < L and Cls[g][lnext] == 0:
                        lnext += 1
                    if lnext < L and pend[g] is None:
                        pend[g] = (emit_prep(g, lnext, offs[g]), lnext)

        stack.close()

    nc.compile()
    return nc


def _in_map_for_core(entry, emb16):
    """Build the device input map for one core from its schedule entry."""
    npdt = emb16.dtype
    m = {}
    for g in range(STREAMS):
        trees = entry[f"trees{g}"]
        eflat = entry[f"eidx{g}"]
        flat = emb16[trees].reshape(TREES_PER_STREAM * N_NODE, DIM)
        xs = np.zeros((len(eflat), DIM), npdt)
        valid = eflat >= 0
        xs[valid] = flat[eflat[valid]]
        if XLAYT:
            # row r=(off+c)*128+p -> [partition p, cols (off+c)*512 ...]
            m[f"xsort{g}"] = np.ascontiguousarray(
                xs.reshape(-1, PART, DIM).transpose(1, 0, 2).reshape(PART, -1))
        else:
            m[f"xsort{g}"] = xs
        if QFUSE:
            # -|x|^2 per state row as sigmoid bias; row r sits at
            # [partition r%128, column r//128]
            q = (xs.astype(np.float32) ** 2).sum(axis=1)
            m[f"negq{g}"] = np.ascontiguousarray(
                (-q).reshape(-1, PART).T.astype(np.float32))
        m[f"pidx{g}"] = entry[f"pidx{g}"]
        if DYN:
            m[f"cnt{g}"] = entry[f"cnt{g}"]
    return m


def kernel(tree_embedding, node_connection, node_mask=None):
    import sys
    if "/opt/trn_rl_repo" not in sys.path:
        sys.path.insert(0, "/opt/trn_rl_repo")
    from concourse.bass_utils import run_bass_kernel_spmd

    emb = np.asarray(tree_embedding, dtype=np.float32)
    conn = np.asarray(node_connection).astype(np.int32)
    B, N, D = emb.shape
    assert D == DIM and B == N_CORES * STREAMS * TREES_PER_STREAM

    L, Cls, sched = _build_schedule(conn)
    nc = _build_program(L, Cls)

    npdt = np.float16 if F16 else np.float32
    emb16 = emb.astype(npdt)
    in_maps = [_in_map_for_core(sched[c], emb16) for c in range(N_CORES)]

    res = run_bass_kernel_spmd(nc, in_maps, list(range(N_CORES)))

    out = np.empty((B, N, DIM), np.float32)
    for c in range(N_CORES):
        for g in range(STREAMS):
            state = np.asarray(res.results[c][f"state{g}"], dtype=np.float32)
            posmat = sched[c][f"posmat{g}"]
            for t, b in enumerate(sched[c][f"trees{g}"]):
                out[b] = state[posmat[t]]
    return out


# revision 5
# speedup vs baseline: 1.1889x; 1.1889x over previous
"""Trainium2 Bass kernel for nn_BareDotProdAttnEncoder (tree scan, gnn_message_passing).

Reference semantics (per batch element b):
  h_0 = x_0
  for i in 1..N-1:
      p = parent[i]  (p < i)
      alpha = exp(<h_p, x_i>); beta = exp(<x_i, x_i>)
      h_i = (alpha*h_p + beta*x_i) / (alpha + beta + 1e-15)

Equivalent form used on device:
  w = sigmoid(<h_p, x_i> - <x_i, x_i>)      (= alpha/(alpha+beta))
  h_i = w*(h_p - x_i) + x_i

Since depth[i] = depth[parent[i]] + 1, all parents of level-l nodes are at
level l-1: the scan is a level-by-level sweep (L ~ 18 levels for N=2048).
The host computes the level schedule, sorts nodes level-contiguously, and
pre-permutes the embeddings into that order, so the device does:
  per level: contiguous X load (prefetchable), SWDGE index-gather of parent
  h from the persistent HBM state buffer, the dot/sigmoid/blend math, and
  contiguous writebacks of the level into the state buffer (which is also
  the kernel output, inverse-permuted host-side).

All device data is fp16 (tolerance 2e-2; fp16 keeps rel err ~1e-3), with
fp32 dot-product accumulation.

Sharding: pure data parallelism over the batch; each of the 8 cores owns
4 trees, processed as STREAMS independent streams that pipeline against
each other.
"""

import os
import numpy as np

N_CORES = 8
STREAMS = int(os.environ.get("K_STREAMS", "2"))
TREES_PER_STREAM = 4 // STREAMS
DIM = 512
PART = 128
N_NODE = 2048
REPEAT = int(os.environ.get("K_REPEAT", "1"))
DYN = os.environ.get("K_DYN", "1") == "1"  # dynamic gather counts (skip pad traffic)
SINGLE_PACKET = os.environ.get("K_SINGLEPKT", "1") == "1"
XBUFS = int(os.environ.get("K_XBUFS", "3"))
HBUFS = int(os.environ.get("K_HBUFS", "2"))
PBUFS = int(os.environ.get("K_PBUFS", "2"))
DBUFS = int(os.environ.get("K_DBUFS", "2"))
TBUFS = int(os.environ.get("K_TBUFS", "2"))
F16 = os.environ.get("K_F16", "1") == "1"
DOT = os.environ.get("K_DOT", "stt")       # stt | altact (alternate stt / Act-accum)
BLEND = os.environ.get("K_BLEND", "act")   # stt | act
QFUSE = os.environ.get("K_QFUSE", "1") == "1"  # host |x|^2 as sigmoid bias
PREP = os.environ.get("K_PREP", "1") == "1"    # prepared+triggered gathers
WBSPLIT = int(os.environ.get("K_WBSPLIT", "2"))  # writebacks per level
STAGGER = os.environ.get("K_STAGGER", "0") == "1"
ABLATE = os.environ.get("K_ABLATE", "")  # nodep: gather from fake tensor
XLAYT = os.environ.get("K_XLAYT", "1") == "1"  # xsort in [128, sumC*512] layout
GRAN = os.environ.get("K_GRAN", "level")  # level | chunk op emission granularity


def _compute_depths(conn):
    B, N = conn.shape
    depths = np.zeros((B, N), np.int32)
    bidx = np.arange(B)
    for i in range(1, N):
        depths[:, i] = depths[bidx, conn[:, i]] + 1
    return depths


def _assign_trees(S, B):
    """Group trees into (stream, core) slots to minimize total padded chunks.
    S: per-tree level-size matrix [B, L]. Returns groups[g][c] = tuple of trees.
    Deterministic local search (seeded)."""
    L = S.shape[1]
    tps = TREES_PER_STREAM
    nslots = B // tps  # STREAMS * N_CORES
    nat = [tuple(range(tps * s, tps * (s + 1))) for s in range(nslots)]
    if tps == 4:
        return [[nat[g * N_CORES + c] for c in range(N_CORES)] for g in range(STREAMS)]

    def cost(assign):
        tot = 0
        for g in range(STREAMS):
            lv = np.zeros(L, np.int64)
            for c in range(N_CORES):
                grp = assign[g * N_CORES + c]
                n = np.sum(S[list(grp)], axis=0)
                lv = np.maximum(lv, (n + PART - 1) // PART)
            tot += lv.sum()
        return int(tot)

    rng = np.random.default_rng(12345)
    cur = [list(p) for p in nat]
    cc = cost([tuple(p) for p in cur])
    best, bc = [tuple(p) for p in cur], cc
    for _ in range(20000):
        a = int(rng.integers(0, nslots)); b2 = int(rng.integers(0, nslots))
        if a == b2:
            continue
        i = int(rng.integers(0, tps)); j = int(rng.integers(0, tps))
        cur[a][i], cur[b2][j] = cur[b2][j], cur[a][i]
        c2 = cost([tuple(p) for p in cur])
        if c2 <= cc:
            cc = c2
            if c2 < bc:
                best, bc = [tuple(p) for p in cur], c2
        else:
            cur[a][i], cur[b2][j] = cur[b2][j], cur[a][i]
    return [[best[g * N_CORES + c] for c in range(N_CORES)] for g in range(STREAMS)]


def _build_schedule(conn):
    """Host-side schedule: level structure, per-core index arrays, maps.

    Returns (L, Cls, sched) where
      L: number of levels
      Cls[g]: list of per-level chunk counts (uniform across cores)
      sched[c]: dict with per-core input arrays + posmat for assembly
    """
    B, N = conn.shape
    depths = _compute_depths(conn)
    L = int(depths.max()) + 1

    # node lists per (batch, level), ordered by node id (stable)
    order = [[np.nonzero(depths[b] == l)[0] for l in range(L)] for b in range(B)]

    S = np.zeros((B, L), np.int64)
    for b in range(B):
        S[b] = np.bincount(depths[b], minlength=L)
    groups = _assign_trees(S, B)  # groups[g][c] = tree tuple

    # uniform chunk capacities per stream
    Cls = []
    for g in range(STREAMS):
        Cl = np.zeros(L, np.int64)
        for c in range(N_CORES):
            trees = groups[g][c]
            for l in range(L):
                n = sum(len(order[b][l]) for b in trees)
                Cl[l] = max(Cl[l], (n + PART - 1) // PART)
        Cls.append([int(x) for x in Cl])

    sched = []
    for c in range(N_CORES):
        entry = {}
        for g in range(STREAMS):
            Cl = Cls[g]
            sumC = sum(Cl)
            R = PART * sumC
            trees = groups[g][c]
            pad = np.int16(-1 if DYN else 0)
            eidx = np.full(R, -1, np.int32)     # row -> embedding row (t*N + i)
            pidx = np.full(R, pad, np.int16)    # row -> parent state row
            cnt = np.zeros(L, np.int32)         # real rows per level (min 1)
            posmat = np.zeros((TREES_PER_STREAM, N), np.int32)  # node -> state row
            off = 0
            for l in range(L):
                base = PART * off
                j = 0
                for t, b in enumerate(trees):
                    for i in order[b][l]:
                        row = base + j
                        eidx[row] = t * N + i
                        posmat[t, i] = row
                        if l > 0:
                            pidx[row] = posmat[t, conn[b, i]]
                        j += 1
                assert j <= PART * Cl[l]
                if j == 0 and Cl[l] > 0:
                    pidx[base] = 0
                    j = 1
                cnt[l] = j
                off += Cl[l]

            def wrap(vals):
                # gather index layout: within a call of num_idxs n, index j
                # lives at [j%16, j//16]; replicate across the 8 groups of
                # 16 partitions. Calls slice per-level column blocks.
                out = np.zeros((PART, 8 * sumC), np.int16)
                o = 0
                for l in range(L):
                    n = PART * Cl[l]
                    block = vals[PART * o : PART * o + n].reshape(8 * Cl[l], 16).T
                    for rep in range(8):
                        out[16 * rep : 16 * (rep + 1), 8 * o : 8 * (o + Cl[l])] = block
                    o += Cl[l]
                return out

            entry[f"pidx{g}"] = wrap(pidx)
            entry[f"eidx{g}"] = eidx  # flat, for host-side emb permutation
            entry[f"cnt{g}"] = cnt.reshape(1, L)
            entry[f"posmat{g}"] = posmat
            entry[f"trees{g}"] = list(trees)
        sched.append(entry)
    return L, Cls, sched


def _build_program(L, Cls):
    import concourse.bacc as bacc
    import concourse.mybir as mybir
    import concourse.tile as tile

    fdt = mybir.dt.float16 if F16 else mybir.dt.float32
    f32 = mybir.dt.float32
    i16 = mybir.dt.int16
    i32 = mybir.dt.int32
    Alu = mybir.AluOpType
    Act = mybir.ActivationFunctionType

    nqueues = min(STREAMS, 4) if PREP else 1
    nc = bacc.Bacc("TRN2", debug=False, num_swdge_queues=nqueues)

    xsort_t, pidx_t, cnt_t, state_t, negq_t = [], [], [], [], []
    for g in range(STREAMS):
        sumC = sum(Cls[g])
        R = PART * sumC
        xshape = [PART, sumC * DIM] if XLAYT else [R, DIM]
        xsort_t.append(nc.dram_tensor(f"xsort{g}", xshape, fdt,
                                      kind="ExternalInput"))
        pidx_t.append(nc.dram_tensor(f"pidx{g}", [PART, 8 * sumC], i16,
                                     kind="ExternalInput"))
        cnt_t.append(nc.dram_tensor(f"cnt{g}", [1, L], i32, kind="ExternalInput"))
        if QFUSE:
            negq_t.append(nc.dram_tensor(f"negq{g}", [PART, sumC], f32,
                                         kind="ExternalInput"))
        state_t.append(nc.dram_tensor(f"state{g}", [R, DIM], fdt,
                                      kind="ExternalOutput"))
    fake_t = None
    if ABLATE == "nodep":
        Rmax = max(PART * sum(C) for C in Cls)
        fake_t = nc.dram_tensor("fake", [Rmax, DIM], fdt)

    with tile.TileContext(nc) as tc:
        from contextlib import ExitStack
        stack = ExitStack()
        pools = []
        for g in range(STREAMS):
            p = {
                "X": stack.enter_context(tc.tile_pool(name=f"X{g}", bufs=XBUFS)),
                "P": stack.enter_context(tc.tile_pool(name=f"P{g}", bufs=PBUFS)),
                "D": stack.enter_context(tc.tile_pool(name=f"D{g}", bufs=DBUFS)),
                "T": stack.enter_context(tc.tile_pool(name=f"T{g}", bufs=TBUFS)),
                "H": stack.enter_context(tc.tile_pool(name=f"H{g}", bufs=HBUFS)),
                "M": stack.enter_context(tc.tile_pool(name=f"M{g}", bufs=2)),
                "S": stack.enter_context(tc.tile_pool(name=f"S{g}", bufs=2)),
                "I": stack.enter_context(tc.tile_pool(name=f"I{g}", bufs=1)),
            }
            pools.append(p)

        # preload index arrays, allocate junk tiles
        idxs = []
        for g in range(STREAMS):
            sumC = sum(Cls[g])
            pi = pools[g]["I"].tile([PART, 8 * sumC], i16, tag=f"pi{g}")
            nq = None
            if QFUSE:
                nq = pools[g]["I"].tile([PART, sumC], f32, tag=f"nq{g}")
                nc.sync.dma_start(nq[:, :], negq_t[g][:, :])
            jt = pools[g]["I"].tile([PART, DIM], fdt, tag=f"jt{g}")   # dot junk out
            ja = pools[g]["I"].tile([PART, DIM], fdt, tag=f"ja{g}")   # act junk out
            nc.sync.dma_start(pi[:, :], pidx_t[g][:, :])
            cr = None
            if DYN:
                ct = pools[g]["I"].tile([1, L], i32, tag=f"ct{g}")
                nc.sync.dma_start(ct[:, :], cnt_t[g][:, :])
                # one register per level: reusing one would be a WAR hazard
                # under Tile reordering (gather reads reg at exec time)
                regs = [nc.gpsimd.alloc_register(f"cnt{g}_{l}") for l in range(L)]
                cr = (ct, regs)
            sems = None
            if PREP:
                sems = [nc.alloc_semaphore(f"gsem{g}_{l}") for l in range(L)]
            idxs.append((pi, nq, jt, ja, cr, sems))

        for _rep in range(REPEAT):
            offs = [0 for _ in range(STREAMS)]
            if STAGGER:
                emit_order = []
                for w in range(L + STREAMS - 1):
                    for g in range(STREAMS):
                        l = w - g
                        if 0 <= l < L:
                            emit_order.append((l, g))
            else:
                emit_order = [(l, g) for l in range(L) for g in range(STREAMS)]

            # per-stream pending prepared-gather (P tile, level)
            pend = [None for _ in range(STREAMS)]

            def load_nreg(g, l):
                pi, nq, jt, ja, cr, sems = idxs[g]
                if DYN:
                    ct, regs = cr
                    nc.gpsimd.reg_load(regs[l], ct[0:1, l : l + 1])
                    return regs[l]
                return PART * Cls[g][l]

            def emit_prep(g, l, off):
                C = Cls[g][l]
                pi, nq, jt, ja, cr, sems = idxs[g]
                nreg = load_nreg(g, l)
                P = pools[g]["P"].tile([PART, C, DIM], fdt, tag=f"P{g}")
                nc.gpsimd.dma_gather(
                    P[:, :, :], state_t[g][:, :],
                    pi[:, 8 * off : 8 * (off + C)], PART * C, nreg, DIM,
                    single_packet=SINGLE_PACKET,
                    prepare_only=True, sem=sems[l], queue_num=g % 4)
                return P

            for l, g in emit_order:
                C = Cls[g][l]
                if C == 0:
                    continue
                off = offs[g]
                offs[g] += C
                pi, nq, jt, ja, cr, sems = idxs[g]
                p = pools[g]
                n = PART * C

                X = p["X"].tile([PART, C, DIM], fdt, tag=f"X{g}")
                H = p["H"].tile([PART, C, DIM], fdt, tag=f"H{g}")

                if XLAYT:
                    xsrc = xsort_t[g][:, DIM * off : DIM * (off + C)].rearrange(
                        "p (c e) -> p c e", e=DIM)
                else:
                    xsrc = xsort_t[g][PART * off : PART * (off + C)].rearrange(
                        "(c p) e -> p c e", p=PART)
                nc.sync.dma_start(X[:, :, :], xsrc)

                if l == 0:
                    nc.scalar.activation(H[:, :, :], X[:, :, :], Act.Copy)
                else:
                    if PREP:
                        assert pend[g] is not None and pend[g][1] == l
                        P = pend[g][0]
                        pend[g] = None
                        nc.gpsimd.trigger_dma(count=None, queue_num=g % 4)
                    else:
                        nreg = load_nreg(g, l)
                        P = p["P"].tile([PART, C, DIM], fdt, tag=f"P{g}")
                        gsrc = fake_t if ABLATE == "nodep" else state_t[g]
                        nc.gpsimd.dma_gather(
                            P[:, :, :], gsrc[:, :],
                            pi[:, 8 * off : 8 * (off + C)], n, nreg, DIM,
                            single_packet=SINGLE_PACKET)

                    D = p["D"].tile([PART, C, DIM], fdt, tag=f"D{g}")
                    T = None
                    if BLEND == "act":
                        T = p["T"].tile([PART, C, DIM], fdt, tag=f"T{g}")
                    dp = p["S"].tile([PART, C], f32, tag=f"dp{g}")
                    wh = p["S"].tile([PART, C], f32, tag=f"wh{g}")

                    def emit_sub(k=None):
                        sl = slice(None) if k is None else slice(k, k + 1)
                        nc.vector.tensor_tensor(D[:, sl, :], P[:, sl, :],
                                                X[:, sl, :], Alu.subtract)

                    def emit_dot(k):
                        # z_k: QFUSE -> <x, h_p> (q in sigmoid bias);
                        #      else  -> <x, h_p - x>
                        din = P[:, k, :] if QFUSE else D[:, k, :]
                        use_act = (DOT == "altact") and (k % 2 == 1)
                        if use_act:
                            M = p["M"].tile([PART, DIM], fdt, tag=f"M{g}")
                            nc.vector.tensor_tensor(M[:, :], X[:, k, :], din,
                                                    Alu.mult)
                            nc.scalar.activation(ja[:, :], M[:, :], Act.Copy,
                                                 accum_out=dp[:, k : k + 1])
                        else:
                            nc.vector.scalar_tensor_tensor(
                                jt[:, :], X[:, k, :], 0.0, din,
                                Alu.bypass, Alu.mult,
                                accum_out=dp[:, k : k + 1])

                    def emit_sig(k=None):
                        sl = slice(None) if k is None else slice(k, k + 1)
                        if QFUSE:
                            qsl = (slice(off, off + C) if k is None
                                   else slice(off + k, off + k + 1))
                            nc.scalar.activation(wh[:, sl], dp[:, sl], Act.Sigmoid,
                                                 bias=nq[:, qsl])
                        else:
                            nc.scalar.activation(wh[:, sl], dp[:, sl], Act.Sigmoid)

                    def emit_blend(k):
                        if BLEND == "act":
                            # T = w * D (Act), h = T + x (DVE 2x tt)
                            nc.scalar.activation(T[:, k, :], D[:, k, :], Act.Copy,
                                                 scale=wh[:, k : k + 1])
                            nc.vector.tensor_tensor(H[:, k, :], T[:, k, :],
                                                    X[:, k, :], Alu.add)
                        else:
                            # h = w*D + x in one DVE stt (1x)
                            nc.vector.scalar_tensor_tensor(
                                H[:, k, :], D[:, k, :], wh[:, k : k + 1],
                                X[:, k, :], Alu.mult, Alu.add)

                    if GRAN == "level":
                        emit_sub()
                        for k in range(C):
                            emit_dot(k)
                        emit_sig()
                        for k in range(C):
                            emit_blend(k)
                    else:
                        for k in range(C):
                            emit_sub(k)
                            emit_dot(k)
                            emit_sig(k)
                            emit_blend(k)

                # writeback in WBSPLIT pieces so the tail transfer is short
                nsp = min(WBSPLIT, C)
                bounds = [round(i * C / nsp) for i in range(nsp + 1)]
                for i in range(nsp):
                    a, b = bounds[i], bounds[i + 1]
                    if a == b:
                        continue
                    dst = state_t[g][PART * (off + a) : PART * (off + b)].rearrange(
                        "(c p) e -> p c e", p=PART)
                    nc.sync.dma_start(dst, H[:, a:b, :])

                # prep the NEXT level's gather now (desc-gen overlaps this
                # level's DMA/compute; the data dep rides the trigger)
                if PREP:
                    lnext = l + 1
                    while lnext < L and Cls[g][lnext] == 0:
                        lnext += 1
                    if lnext < L and pend[g] is None:
                        pend[g] = (emit_prep(g, lnext, offs[g]), lnext)

        stack.close()

    nc.compile()
    return nc


def _in_map_for_core(entry, emb16):
    """Build the device input map for one core from its schedule entry."""
    npdt = emb16.dtype
    m = {}
    for g in range(STREAMS):
        trees = entry[f"trees{g}"]
        eflat = entry[f"eidx{g}"]
        flat = emb16[trees].reshape(TREES_PER_STREAM * N_NODE, DIM)
        xs = np.zeros((len(eflat), DIM), npdt)
        valid = eflat >= 0
        xs[valid] = flat[eflat[valid]]
        if XLAYT:
            # row r=(off+c)*128+p -> [partition p, cols (off+c)*512 ...]
            m[f"xsort{g}"] = np.ascontiguousarray(
                xs.reshape(-1, PART, DIM).transpose(1, 0, 2).reshape(PART, -1))
        else:
            m[f"xsort{g}"] = xs
        if QFUSE:
            # -|x|^2 per state row as sigmoid bias; row r sits at
            # [partition r%128, column r//128]
            q = (xs.astype(np.float32) ** 2).sum(axis=1)
            m[f"negq{g}"] = np.ascontiguousarray(
                (-q).reshape(-1, PART).T.astype(np.float32))
        m[f"pidx{g}"] = entry[f"pidx{g}"]
        if DYN:
            m[f"cnt{g}"] = entry[f"cnt{g}"]
    return m


def kernel(tree_embedding, node_connection, node_mask=None):
    import sys
    if "/opt/trn_rl_repo" not in sys.path:
        sys.path.insert(0, "/opt/trn_rl_repo")
    from concourse.bass_utils import run_bass_kernel_spmd

    emb = np.asarray(tree_embedding, dtype=np.float32)
    conn = np.asarray(node_connection).astype(np.int32)
    B, N, D = emb.shape
    assert D == DIM and B == N_CORES * STREAMS * TREES_PER_STREAM

    L, Cls, sched = _build_schedule(conn)
    nc = _build_program(L, Cls)

    npdt = np.float16 if F16 else np.float32
    emb16 = emb.astype(npdt)
    in_maps = [_in_map_for_core(sched[c], emb16) for c in range(N_CORES)]

    res = run_bass_kernel_spmd(nc, in_maps, list(range(N_CORES)))

    out = np.empty((B, N, DIM), np.float32)
    for c in range(N_CORES):
        for g in range(STREAMS):
            state = np.asarray(res.results[c][f"state{g}"], dtype=np.float32)
            posmat = sched[c][f"posmat{g}"]
            for t, b in enumerate(sched[c][f"trees{g}"]):
                out[b] = state[posmat[t]]
    return out
